# revision 14
# baseline (speedup 1.0000x reference)
"""Bass/Trainium2 kernel for nn_MaskedLoss (MSE with bbox-ROI weighting).

Self-contained: hardcodes shapes (4,1,160,160,160) f32/i32, shards across
8 NeuronCores as (batch item, D-half) pairs, runs one SPMD Bass program
with one tiny pairwise AllReduce for the bbox exchange, and combines the
per-core partial sums on the host.

v3: keeps DVE under the DMA roofline by distributing the mask projections
across ACT and PE and using the cumsum-extract trick for the w-box.

  - mask phase: per tile, ACT casts i32->bf16 (its accum gives per-
    partition tile sums -> d-axis sums, since rows of one (p,t) share d);
    PE accumulates 16 h-group column sums (one-hot bf16 matmuls) into
    PSUM across all tiles.  DVE does nothing per tile; two small reduces
    at the end produce s_h [16,10] and v_w [1,160].
  - extrema/bbox exchange: BIG-trick extrema per axis, 7 scalar slots in
    a [1,128] row, pairwise AllReduce(max) between the two halves of each
    batch item.  Fires right after the mask stream.
  - bulk phase: per tile, DVE subtract, ACT Square+accumulate (total
    sums), DVE cumsum scan; the w-box is applied via prefix-sum
    differences extracted at dynamic indices (ACT registers), so nothing
    on the bulk path waits for the collective.  Box row weights
    in_d*in_h expand to the [128,100] row layout via two tiny one-hot PE
    matmuls and weight the extracted prefix differences once at the end.
"""

import os
import sys

import numpy as np

sys.path.insert(0, "/opt/trn_rl_repo")

B, D, H, W = 4, 160, 160, 160
HALF_D = D // 2          # 80 d-slices per core
R = HALF_D * H           # 12800 rows (d,h) per core
KJ = 10                  # rows per partition line in a tile (6400B lines)
NT = R // (128 * KJ)     # 10 tiles per tensor per core
N_CORES = 8
BIG = 1.0e6
W_OUT2 = 0.01            # W_OUT ** 2
EXPAND = 1.2
F = KJ * W               # 1600 free elements per tile partition
CHUNK = 400              # PSUM-bank-sized matmul chunks (4 per tile)

_CACHE: dict = {}


def _build_nc():
    from concourse import bacc, bass, bass_isa, tile
    import concourse.mybir as mybir

    f32 = mybir.dt.float32
    i32 = mybir.dt.int32
    bf16 = mybir.dt.bfloat16
    AX = mybir.AxisListType
    OP = mybir.AluOpType
    AF = mybir.ActivationFunctionType
    RO = bass_isa.ReduceOp

    nc = bacc.Bacc(
        "TRN2", target_bir_lowering=False, debug=False, num_devices=N_CORES
    )

    yp = nc.dram_tensor("yp", [R, W], f32, kind="ExternalInput")
    yt = nc.dram_tensor("yt", [R, W], f32, kind="ExternalInput")
    mk = nc.dram_tensor("mk", [R, W], i32, kind="ExternalInput")
    meta = nc.dram_tensor("meta", [1], f32, kind="ExternalInput")
    out = nc.dram_tensor("out", [2], f32, kind="ExternalOutput")

    ypv = yp.ap().rearrange("(t p j) w -> t p j w", p=128, j=KJ)
    ytv = yt.ap().rearrange("(t p j) w -> t p j w", p=128, j=KJ)
    mkv = mk.ap().rearrange("(t p j) w -> t p j w", p=128, j=KJ)

    with tile.TileContext(nc) as tc:
        with (
            tc.tile_pool(name="dram", bufs=1, space="DRAM") as dpool,
            tc.tile_pool(name="persist", bufs=1) as pp,
            tc.tile_pool(name="mkp", bufs=5) as mkp,
            tc.tile_pool(name="mbp", bufs=3) as mbp,
            tc.tile_pool(name="pp2", bufs=6) as ppool,
            tc.tile_pool(name="tp2", bufs=6) as tpool,
            tc.tile_pool(name="psp", bufs=1,
                         space=bass.MemorySpace.PSUM) as pspool,
            tc.tile_pool(name="sqp", bufs=3) as sqpool,
            tc.tile_pool(name="csp", bufs=NT) as cspool,
        ):
            cc1_in = dpool.tile([128], f32, tag="cc1_in")
            cc1_out = dpool.tile([128], f32, tag="cc1_out")

            from concourse.tile_rust import add_dep_helper

            # ---------------- setup: iotas / one-hot weights -------------
            # w16b [128,16] bf16: one-hot of p%16 (h-groups, PE lhsT)
            a_h = pp.tile([128, 16], i32, tag="a_h")
            nc.gpsimd.iota(a_h[:], pattern=[[-1, 16]], base=0,
                           channel_multiplier=1)          # p - m
            a_h_m = pp.tile([128, 16], i32, tag="a_h_m")
            nc.vector.tensor_scalar(out=a_h_m[:], in0=a_h[:], scalar1=15,
                                    scalar2=None, op0=OP.bitwise_and)
            w16b = pp.tile([128, 16], bf16, tag="w16b")
            nc.vector.tensor_scalar(out=w16b[:], in0=a_h_m[:], scalar1=0,
                                    scalar2=None, op0=OP.is_equal)
            # w8d [128,8] f32: one-hot of p//16 (d-groups, on tile sums)
            a_d = pp.tile([128, 8], i32, tag="a_d")
            nc.gpsimd.iota(a_d[:], pattern=[[-16, 8]], base=0,
                           channel_multiplier=1)          # p - 16m
            ts1 = pp.tile([128, 8], f32, tag="ts1")
            nc.vector.tensor_scalar(out=ts1[:], in0=a_d[:], scalar1=-1,
                                    scalar2=None, op0=OP.is_gt)
            ts2 = pp.tile([128, 8], f32, tag="ts2")
            nc.vector.tensor_scalar(out=ts2[:], in0=a_d[:], scalar1=15,
                                    scalar2=None, op0=OP.is_le)
            w8d = pp.tile([128, 8], f32, tag="w8d")
            nc.vector.tensor_tensor(out=w8d[:], in0=ts1[:], in1=ts2[:],
                                    op=OP.mult)

            # E_h [16,128]: one-hot q == p%16 ; E_d [8,128]: q == p//16
            e_h_i = pp.tile([16, 128], i32, tag="e_h_i")
            nc.gpsimd.iota(e_h_i[:], pattern=[[-1, 128]], base=0,
                           channel_multiplier=1)          # q - p
            e_h_m = pp.tile([16, 128], i32, tag="e_h_m")
            nc.vector.tensor_scalar(out=e_h_m[:], in0=e_h_i[:], scalar1=15,
                                    scalar2=None, op0=OP.bitwise_and)
            e_h = pp.tile([16, 128], f32, tag="e_h")
            nc.vector.tensor_scalar(out=e_h[:], in0=e_h_m[:], scalar1=0,
                                    scalar2=None, op0=OP.is_equal)
            e_d_i = pp.tile([8, 128], i32, tag="e_d_i")
            nc.gpsimd.iota(e_d_i[:], pattern=[[-1, 128]], base=0,
                           channel_multiplier=16)         # 16q - p
            td1 = pp.tile([8, 128], f32, tag="td1")
            nc.vector.tensor_scalar(out=td1[:], in0=e_d_i[:], scalar1=-16,
                                    scalar2=None, op0=OP.is_gt)
            td2 = pp.tile([8, 128], f32, tag="td2")
            nc.vector.tensor_scalar(out=td2[:], in0=e_d_i[:], scalar1=0,
                                    scalar2=None, op0=OP.is_le)
            e_d = pp.tile([8, 128], f32, tag="e_d")
            nc.vector.tensor_tensor(out=e_d[:], in0=td1[:], in1=td2[:],
                                    op=OP.mult)

            # index rows for extrema: w [1,160], h [16,10], d [8,10]
            iota_w = pp.tile([1, W], i32, tag="iota_w")
            nc.gpsimd.iota(iota_w[:], pattern=[[1, W]], base=0,
                           channel_multiplier=0)
            k160 = pp.tile([1, W], f32, tag="k160")
            nc.vector.tensor_copy(out=k160[:], in_=iota_w[:])
            iota_h_i = pp.tile([16, KJ], i32, tag="iota_h_i")
            nc.gpsimd.iota(iota_h_i[:], pattern=[[1, KJ]], base=0,
                           channel_multiplier=KJ)         # h = 10q + j
            kh = pp.tile([16, KJ], f32, tag="kh")
            nc.vector.tensor_copy(out=kh[:], in_=iota_h_i[:])
            iota_d_i = pp.tile([8, NT], i32, tag="iota_d_i")
            nc.gpsimd.iota(iota_d_i[:], pattern=[[8, NT]], base=0,
                           channel_multiplier=1)          # d_loc = 8t + q
            meta_s = pp.tile([1, 1], f32, tag="meta_s")
            nc.gpsimd.dma_start(
                out=meta_s[:], in_=meta.ap().rearrange("(p x) -> p x", p=1))
            meta_b8 = pp.tile([8, 1], f32, tag="meta_b8")
            nc.gpsimd.partition_broadcast(meta_b8[:], meta_s[:], channels=8)
            kd = pp.tile([8, NT], f32, tag="kd")
            nc.vector.tensor_copy(out=kd[:], in_=iota_d_i[:])
            nc.vector.tensor_scalar(out=kd[:], in0=kd[:], scalar1=meta_b8[:],
                                    scalar2=None, op0=OP.add)  # global d

            # ---------------- phase 1: mask projections -----------------
            tilesum = pp.tile([128, NT], f32, tag="tilesum")
            psum_h = []
            for c in range(4):
                psh_c = pspool.tile([128, 512], f32, tag=f"psum_h{c}")
                psum_h.append(psh_c)

            mask_dmas = []
            for t in range(NT):
                m_t = mkp.tile([128, F], i32, tag="m_t")
                if t % 2 == 0:
                    dma = nc.sync.dma_start(out=m_t[:], in_=mkv[t])
                else:
                    dma = nc.scalar.dma_start(out=m_t[:], in_=mkv[t])
                mask_dmas.append(dma)
                mb_t = mbp.tile([128, F], bf16, tag="mb_t")
                # cast to bf16 (0/1 exact); accum gives per-(p,t) sums,
                # i.e. d-axis sums since all rows of one (p,t) share d
                nc.scalar.activation(out=mb_t[:], in_=m_t[:], func=AF.Copy,
                                     accum_out=tilesum[:, t : t + 1])
                for c in range(4):
                    nc.tensor.matmul(
                        psum_h[c][:16, :CHUNK], w16b[:],
                        mb_t[:, c * CHUNK : (c + 1) * CHUNK],
                        start=(t == 0), stop=(t == NT - 1))
            mask_sync_last = mask_dmas[NT - 2]
            mask_scal_last = mask_dmas[NT - 1]

            # h-group column sums [16, (j w)] -> s_h [16,10], v_w [1,160]
            sh16 = pp.tile([16, F], f32, tag="sh16")
            for c in range(4):
                nc.scalar.activation(
                    out=sh16[:, c * CHUNK : (c + 1) * CHUNK],
                    in_=psum_h[c][:16, :CHUNK], func=AF.Copy)
            s_h = pp.tile([16, KJ], f32, tag="s_h")
            junk16 = pp.tile([16, W], f32, tag="junk16")
            for j in range(KJ):
                nc.scalar.activation(
                    out=junk16[:], in_=sh16[:, j * W : (j + 1) * W],
                    func=AF.Copy, accum_out=s_h[:, j : j + 1])
            vacc = pp.tile([16, W], f32, tag="vacc")
            nc.gpsimd.memset(vacc[:], 0.0)
            for j in range(KJ):
                nc.gpsimd.tensor_tensor(out=vacc[:], in0=vacc[:],
                                        in1=sh16[:, j * W : (j + 1) * W],
                                        op=OP.add)
            vwr = pp.tile([16, W], f32, tag="vwr")
            nc.gpsimd.partition_all_reduce(vwr[:], vacc[:], channels=16,
                                           reduce_op=RO.add)
            v_w = vwr[0:1, :]

            ps_d24 = pspool.tile([128, 512], f32, tag="ps_d24")
            nc.tensor.matmul(ps_d24[:8, :NT], w8d[:], tilesum[:])
            s_d = pp.tile([8, NT], f32, tag="s_d")
            nc.vector.tensor_copy(out=s_d[:], in_=ps_d24[0:8, 0:NT])

            # ---------------- extrema (BIG trick) + CC -------------------
            def extrema(val, idx, p, tagp):
                # returns (gt, -mn|-BIG, mx|-BIG) as [p,1] (row 0 valid)
                n = val.shape[1]
                gt = pp.tile([p, n], f32, tag=f"gt_{tagp}")
                nc.vector.tensor_scalar(out=gt[:], in0=val, scalar1=0.0,
                                        scalar2=None, op0=OP.is_gt)
                bm = pp.tile([p, n], f32, tag=f"bm_{tagp}")
                nc.vector.tensor_scalar(out=bm[:], in0=idx, scalar1=-1.0,
                                        scalar2=BIG, op0=OP.mult, op1=OP.add)
                kp = pp.tile([p, n], f32, tag=f"kp_{tagp}")
                nc.vector.tensor_scalar(out=kp[:], in0=idx, scalar1=BIG,
                                        scalar2=None, op0=OP.add)
                ta = pp.tile([p, n], f32, tag=f"ta_{tagp}")
                nc.vector.tensor_tensor(out=ta[:], in0=gt[:], in1=bm[:],
                                        op=OP.mult)
                ra = pp.tile([p, 1], f32, tag=f"ra_{tagp}")
                nc.vector.tensor_reduce(out=ra[:], in_=ta[:], axis=AX.X,
                                        op=OP.max)
                tb = pp.tile([p, n], f32, tag=f"tb_{tagp}")
                nc.vector.tensor_tensor(out=tb[:], in0=gt[:], in1=kp[:],
                                        op=OP.mult)
                rb = pp.tile([p, 1], f32, tag=f"rb_{tagp}")
                nc.vector.tensor_reduce(out=rb[:], in_=tb[:], axis=AX.X,
                                        op=OP.max)
                if p > 1:
                    ra2 = pp.tile([p, 1], f32, tag=f"ra2_{tagp}")
                    nc.gpsimd.partition_all_reduce(ra2[:], ra[:], channels=p,
                                                   reduce_op=RO.max)
                    rb2 = pp.tile([p, 1], f32, tag=f"rb2_{tagp}")
                    nc.gpsimd.partition_all_reduce(rb2[:], rb[:], channels=p,
                                                   reduce_op=RO.max)
                    return gt, ra2, rb2
                return gt, ra, rb

            gt_w, ra_w, rb_w = extrema(v_w, k160[:], 1, "w")
            _, ra_h, rb_h = extrema(s_h[:], kh[:], 16, "h")
            _, ra_d, rb_d = extrema(s_d[:], kd[:], 8, "d")
            hf_sum = pp.tile([1, 1], f32, tag="hf_sum")
            nc.vector.tensor_reduce(out=hf_sum[:], in_=gt_w[:], axis=AX.X,
                                    op=OP.add)
            hf_loc = pp.tile([1, 1], f32, tag="hf_loc")
            nc.vector.tensor_scalar(out=hf_loc[:], in0=hf_sum[:], scalar1=0.0,
                                    scalar2=None, op0=OP.is_gt)

            p8w = pp.tile([1, 128], f32, tag="p8w")
            nc.vector.memset(p8w[:], 0.0)
            for slot, src in ((0, ra_w[:]), (1, rb_w[:]),
                              (3, ra_d[0:1, 0:1]), (4, rb_d[0:1, 0:1]),
                              (5, ra_h[0:1, 0:1]), (6, rb_h[0:1, 0:1])):
                nc.vector.tensor_scalar(out=p8w[:, slot : slot + 1], in0=src,
                                        scalar1=-BIG, scalar2=None,
                                        op0=OP.add)
            nc.vector.tensor_copy(out=p8w[:, 2:3], in_=hf_loc[:])
            nc.gpsimd.dma_start(
                out=cc1_in[:].rearrange("(p x) -> p x", p=1), in_=p8w[:])
            nc.gpsimd.collective_compute(
                "AllReduce", OP.max,
                replica_groups=[[0, 1], [2, 3], [4, 5], [6, 7]],
                ins=[cc1_in[:].opt()], outs=[cc1_out[:].opt()])
            g8w = pp.tile([1, 128], f32, tag="g8w")
            nc.gpsimd.dma_start(
                out=g8w[:], in_=cc1_out[:].rearrange("(p x) -> p x", p=1))

            # ---------------- box bounds ------------------
            def centers(slot_mn, slot_mx, tagp):
                # returns (c, e) in f32 exactly as the reference computes
                mn = pp.tile([1, 1], f32, tag=f"mn_{tagp}")
                nc.vector.tensor_scalar(out=mn[:],
                                        in0=g8w[:, slot_mn : slot_mn + 1],
                                        scalar1=-1.0, scalar2=None,
                                        op0=OP.mult)
                mx = g8w[:, slot_mx : slot_mx + 1]
                c2 = pp.tile([1, 1], f32, tag=f"c2_{tagp}")
                nc.vector.tensor_tensor(out=c2[:], in0=mn[:], in1=mx,
                                        op=OP.add)
                cC = pp.tile([1, 1], f32, tag=f"cC_{tagp}")
                nc.vector.tensor_scalar(out=cC[:], in0=c2[:], scalar1=0.5,
                                        scalar2=None, op0=OP.mult)
                em = pp.tile([1, 1], f32, tag=f"em_{tagp}")
                nc.vector.tensor_tensor(out=em[:], in0=mx, in1=mn[:],
                                        op=OP.subtract)
                nc.vector.tensor_scalar(out=em[:], in0=em[:], scalar1=1.0,
                                        scalar2=0.5, op0=OP.add, op1=OP.mult)
                eE = pp.tile([1, 1], f32, tag=f"eE_{tagp}")
                nc.vector.tensor_scalar(out=eE[:], in0=em[:], scalar1=EXPAND,
                                        scalar2=None, op0=OP.mult)
                return cC, eE

            def bounds(cC, eE, tagp):
                # (lo-1, min(hi-1, W-2)) for is_gt/is_le compare form
                lo = pp.tile([1, 1], f32, tag=f"lo_{tagp}")
                nc.vector.tensor_tensor(out=lo[:], in0=cC[:], in1=eE[:],
                                        op=OP.subtract)
                nc.vector.tensor_scalar(out=lo[:], in0=lo[:], scalar1=-1.0,
                                        scalar2=None, op0=OP.add)
                hi = pp.tile([1, 1], f32, tag=f"hi_{tagp}")
                nc.vector.tensor_tensor(out=hi[:], in0=cC[:], in1=eE[:],
                                        op=OP.add)
                nc.vector.tensor_scalar(out=hi[:], in0=hi[:], scalar1=-1.0,
                                        scalar2=float(W - 2), op0=OP.add,
                                        op1=OP.min)
                return lo, hi

            cw, ew = centers(0, 1, "w")
            cd, ed = centers(3, 4, "d")
            ch, eh = centers(5, 6, "h")
            lo_d, hi_d = bounds(cd, ed, "d")
            lo_h, hi_h = bounds(ch, eh, "h")

            # w box -> integer cumsum-extract indices RA/RB (ACT registers)
            cpe = pp.tile([1, 1], f32, tag="cpe")
            nc.vector.tensor_tensor(out=cpe[:], in0=cw[:], in1=ew[:],
                                    op=OP.add)
            cme = pp.tile([1, 1], f32, tag="cme")
            nc.vector.tensor_tensor(out=cme[:], in0=cw[:], in1=ew[:],
                                    op=OP.subtract)

            def floor_clamp_idx(x, tagp):
                # cast-mode-agnostic floor, clamped to [0, W-1], as int32
                yi = pp.tile([1, 1], i32, tag=f"yi_{tagp}")
                nc.vector.tensor_copy(out=yi[:], in_=x)
                yf = pp.tile([1, 1], f32, tag=f"yf_{tagp}")
                nc.vector.tensor_copy(out=yf[:], in_=yi[:])
                corr = pp.tile([1, 1], f32, tag=f"corr_{tagp}")
                nc.vector.tensor_tensor(out=corr[:], in0=yf[:], in1=x,
                                        op=OP.is_gt)
                fl = pp.tile([1, 1], f32, tag=f"fl_{tagp}")
                nc.vector.tensor_tensor(out=fl[:], in0=yf[:], in1=corr[:],
                                        op=OP.subtract)
                nc.vector.tensor_scalar(out=fl[:], in0=fl[:], scalar1=0.0,
                                        scalar2=float(W - 1), op0=OP.max,
                                        op1=OP.min)
                ii = pp.tile([1, 1], i32, tag=f"ii_{tagp}")
                nc.vector.tensor_copy(out=ii[:], in_=fl[:])
                return ii

            ra_i = floor_clamp_idx(cpe[:], "ra")
            rb_i = floor_clamp_idx(cme[:], "rb")

            # h-box [16,10] and d-box [8,10] rows, then expand to [128,*]
            lo_h_b = pp.tile([16, 1], f32, tag="lo_h_b")
            nc.gpsimd.partition_broadcast(lo_h_b[:], lo_h[:], channels=16)
            hi_h_b = pp.tile([16, 1], f32, tag="hi_h_b")
            nc.gpsimd.partition_broadcast(hi_h_b[:], hi_h[:], channels=16)
            ga_h = pp.tile([16, KJ], f32, tag="ga_h")
            nc.vector.tensor_scalar(out=ga_h[:], in0=kh[:], scalar1=lo_h_b[:],
                                    scalar2=None, op0=OP.is_gt)
            la_h = pp.tile([16, KJ], f32, tag="la_h")
            nc.vector.tensor_scalar(out=la_h[:], in0=kh[:], scalar1=hi_h_b[:],
                                    scalar2=None, op0=OP.is_le)
            in_h = pp.tile([16, KJ], f32, tag="in_h")
            nc.vector.tensor_tensor(out=in_h[:], in0=ga_h[:], in1=la_h[:],
                                    op=OP.mult)

            lo_d_b = pp.tile([8, 1], f32, tag="lo_d_b")
            nc.gpsimd.partition_broadcast(lo_d_b[:], lo_d[:], channels=8)
            hi_d_b = pp.tile([8, 1], f32, tag="hi_d_b")
            nc.gpsimd.partition_broadcast(hi_d_b[:], hi_d[:], channels=8)
            hf_b = pp.tile([8, 1], f32, tag="hf_b")
            nc.gpsimd.partition_broadcast(hf_b[:], g8w[:, 2:3], channels=8)
            ga_d = pp.tile([8, NT], f32, tag="ga_d")
            nc.vector.tensor_scalar(out=ga_d[:], in0=kd[:], scalar1=lo_d_b[:],
                                    scalar2=None, op0=OP.is_gt)
            la_d = pp.tile([8, NT], f32, tag="la_d")
            nc.vector.tensor_scalar(out=la_d[:], in0=kd[:], scalar1=hi_d_b[:],
                                    scalar2=None, op0=OP.is_le)
            in_d = pp.tile([8, NT], f32, tag="in_d")
            nc.vector.tensor_tensor(out=in_d[:], in0=ga_d[:], in1=la_d[:],
                                    op=OP.mult)
            nc.vector.tensor_scalar(out=in_d[:], in0=in_d[:],
                                    scalar1=hf_b[:], scalar2=None,
                                    op0=OP.mult)  # fold has_fg once

            ps_h = pspool.tile([128, 512], f32, tag="ps_h")
            nc.tensor.matmul(ps_h[:128, :KJ], e_h[:], in_h[:])
            inh_pj = pp.tile([128, KJ], f32, tag="inh_pj")
            nc.vector.tensor_copy(out=inh_pj[:], in_=ps_h[:, 0:KJ])
            ps_d = pspool.tile([128, 512], f32, tag="ps_d")
            nc.tensor.matmul(ps_d[:128, :NT], e_d[:], in_d[:])
            ind_pt = pp.tile([128, NT], f32, tag="ind_pt")
            nc.vector.tensor_copy(out=ind_pt[:], in_=ps_d[:, 0:NT])
            wdh = pp.tile([128, NT * KJ], f32, tag="wdh")
            for t in range(NT):
                nc.vector.tensor_scalar(
                    out=wdh[:, t * KJ : (t + 1) * KJ], in0=inh_pj[:],
                    scalar1=ind_pt[:, t : t + 1], scalar2=None, op0=OP.mult)

            # ---------------- phase 2: weighted MSE sums ----------------
            acc_tot = pp.tile([128, NT], f32, tag="acc_tot")
            acc_a = pp.tile([128, NT * KJ], f32, tag="acc_a")
            acc_b = pp.tile([128, NT * KJ], f32, tag="acc_b")

            # issue pass: all bulk dma_starts before any gated compute
            bulk_tiles = []
            for t in range(NT):
                p_t = ppool.tile([128, F], f32, tag="p_t")
                yp_dma = nc.sync.dma_start(out=p_t[:], in_=ypv[t])
                t_t = tpool.tile([128, F], f32, tag="t_t")
                yt_dma = nc.scalar.dma_start(out=t_t[:], in_=ytv[t])
                if t == 0:
                    add_dep_helper(yp_dma.ins, mask_sync_last.ins, sync=False,
                                   reason="mask first on sync queue")
                    add_dep_helper(yp_dma.ins, mask_scal_last.ins, sync=True,
                                   reason="mask first (cross queue)")
                    add_dep_helper(yt_dma.ins, mask_scal_last.ins, sync=False,
                                   reason="mask first on scalar queue")
                    add_dep_helper(yt_dma.ins, mask_sync_last.ins, sync=True,
                                   reason="mask first (cross queue)")
                bulk_tiles.append((p_t, t_t))

            # regs load after square 7 in the ACT stream: late enough
            # that the collective is done (no stall), early enough that
            # most extracts overlap the remaining bulk stream
            rav = rbv = None
            cs_tiles = []

            def extract(t):
                cs_t = cs_tiles[t]
                nc.scalar.activation(
                    out=acc_a[:, t * KJ : (t + 1) * KJ],
                    in_=cs_t[:, bass.ds(rav, KJ, W)], func=AF.Copy)
                nc.scalar.activation(
                    out=acc_b[:, t * KJ : (t + 1) * KJ],
                    in_=cs_t[:, bass.ds(rbv, KJ, W)], func=AF.Copy)

            for t in range(NT):
                p_t, t_t = bulk_tiles[t]
                nc.vector.tensor_tensor(out=p_t[:], in0=p_t[:],
                                        in1=t_t[:], op=OP.subtract)
                sq_t = sqpool.tile([128, F], bf16, tag="sq_t")
                nc.scalar.activation(
                    out=sq_t[:], in_=p_t[:], func=AF.Square,
                    accum_out=acc_tot[:, t : t + 1])
                cs_t = cspool.tile([128, F + 1], bf16, tag="cs_t")
                nc.vector.memset(cs_t[:, 0:1], 0.0)
                with nc.allow_low_precision("bf16 prefix sums: box-sum "
                                            "error ~4e-4 rel, gate is 2e-2"):
                    nc.vector.tensor_tensor_scan(
                        out=cs_t[:, 1 : F + 1], data0=sq_t[:], data1=sq_t[:],
                        initial=0.0, op0=OP.add, op1=OP.bypass)
                cs_tiles.append(cs_t)
                if t == 7:
                    reg_ra = nc.alloc_register(nc.scalar.engine, "reg_ra")
                    nc.scalar.reg_load(reg_ra, ra_i[0:1, 0:1])
                    rav = nc.scalar.snap(reg_ra, min_val=0, max_val=W - 1)
                    reg_rb = nc.alloc_register(nc.scalar.engine, "reg_rb")
                    nc.scalar.reg_load(reg_rb, rb_i[0:1, 0:1])
                    rbv = nc.scalar.snap(reg_rb, min_val=0, max_val=W - 1)
                    for tt in range(8):
                        extract(tt)
                elif t > 7:
                    extract(t)

            # ---------------- final reductions ----------------
            junk_a = pp.tile([128, NT * KJ], f32, tag="junk_a")
            sa_col = pp.tile([128, 1], f32, tag="sa_col")
            nc.vector.tensor_tensor(out=junk_a[:], in0=acc_a[:],
                                    in1=wdh[:], op=OP.mult)
            nc.vector.tensor_reduce(out=sa_col[:], in_=junk_a[:], axis=AX.X,
                                    op=OP.add)
            junk_b = pp.tile([128, NT * KJ], f32, tag="junk_b")
            sb_col = pp.tile([128, 1], f32, tag="sb_col")
            nc.vector.tensor_tensor(out=junk_b[:], in0=acc_b[:],
                                    in1=wdh[:], op=OP.mult)
            nc.vector.tensor_reduce(out=sb_col[:], in_=junk_b[:], axis=AX.X,
                                    op=OP.add)
            box_col = pp.tile([128, 1], f32, tag="box_col")
            nc.vector.tensor_tensor(out=box_col[:], in0=sa_col[:],
                                    in1=sb_col[:], op=OP.subtract)
            tot_col = pp.tile([128, 1], f32, tag="tot_col")
            nc.vector.tensor_reduce(out=tot_col[:], in_=acc_tot[:],
                                    axis=AX.X, op=OP.add)
            pair = pp.tile([128, 2], f32, tag="pair")
            nc.vector.tensor_copy(out=pair[:, 0:1], in_=tot_col[:])
            nc.vector.tensor_copy(out=pair[:, 1:2], in_=box_col[:])
            pr = pp.tile([128, 2], f32, tag="pr")
            nc.gpsimd.partition_all_reduce(pr[:], pair[:], channels=128,
                                           reduce_op=RO.add)
            res2 = pp.tile([1, 2], f32, tag="res2")
            nc.vector.tensor_copy(out=res2[:], in_=pr[0:1, :])
            nc.gpsimd.dma_start(
                out=out.ap().rearrange("(p x) -> p x", p=1), in_=res2[:])

    nc.compile()
    return nc


def get_nc():
    if "nc" not in _CACHE:
        _CACHE["nc"] = _build_nc()
    return _CACHE["nc"]


def make_in_maps(y_pred, y_true, mask):
    y_pred = np.asarray(y_pred, dtype=np.float32).reshape(B, D, H, W)
    y_true = np.asarray(y_true, dtype=np.float32).reshape(B, D, H, W)
    mask = np.asarray(mask, dtype=np.int32).reshape(B, D, H, W)
    in_maps = []
    for c in range(N_CORES):
        b, half = c // 2, c % 2
        sl = slice(half * HALF_D, (half + 1) * HALF_D)
        in_maps.append({
            "yp": np.ascontiguousarray(y_pred[b, sl]).reshape(R, W),
            "yt": np.ascontiguousarray(y_true[b, sl]).reshape(R, W),
            "mk": np.ascontiguousarray(mask[b, sl]).reshape(R, W),
            "meta": np.array([half * HALF_D], dtype=np.float32),
        })
    return in_maps


def combine(results):
    tot = 0.0
    box = 0.0
    for r in results:
        o = np.asarray(r["out"], dtype=np.float64).reshape(-1)
        tot += o[0]
        box += o[1]
    loss = (W_OUT2 * tot + (1.0 - W_OUT2) * box) / float(B * D * H * W)
    return np.array(loss, dtype=np.float32)


def kernel(y_pred, y_true, mask):
    from concourse.bass_utils import run_bass_kernel_spmd

    nc = get_nc()
    in_maps = make_in_maps(y_pred, y_true, mask)
    trace = bool(int(os.environ.get("BASS_KERNEL_TRACE", "0")))
    kwargs = {}
    if trace:
        kwargs = dict(trace=True, trace_cores=[0])
    res = run_bass_kernel_spmd(
        nc, in_maps, core_ids=list(range(N_CORES)), **kwargs
    )
    _CACHE["last_results"] = res
    return combine(res.results)
